# Initial kernel scaffold
#
"""GATv2-style masked attention kernel for Trainium2, 8-core data-parallel over batch.

Per core (one batch element, N=2048 nodes, F=256 features):
  h = x @ W                              (PE, fp32r)
  s_src = h @ a[:F], s_dst = h @ a[F:]   (PE, fused into the same matmuls)
  e[i,j] = leaky_relu(s_src[i] + s_dst[j], 0.2), masked by A
  alpha = softmax_j(e); y = alpha @ h

Softmax without row maxima: any per-i factor (and any global factor) cancels
in the normalization y = (P @ [h|1]) -> y[:, :F] / y[:, F], so we use
  P[j,i] = exp(leaky(u) - s_src_i - 54)
         = exp(max(-0.8*s_src_i, 0.8*s_dst_j) + 0.2*s_dst_j - 54)
with u = s_src_i + s_dst_j. The -54 recenters args near the typical row max
(3.4*sigma with sigma = ||W @ a_dst|| ~= 16 for this randn input spec) so the
fp16 score tiles keep precision where the big softmax weights live; bf16 P and
fp32 PSUM absorb the residual range with no under/overflow for any plausible
draw. The mask is applied multiplicatively after exp.

Scores are built transposed ([j, i]) so the P @ h contraction has j on
partitions. The i range is processed in two waves of 8 PSUM banks each, with
the mask resident in SBUF, so the P@h matmuls fully overlap score production.
The host supplies: x transposed, the mask transposed as bf16 {0,1}, W with the
attention vectors folded in ([W | W@a_src | W@a_dst]), and W@a_src replicated
across 128 columns (pure layout/weight transforms of the inputs).
"""

import numpy as np

B, N, F = 8, 2048, 256
PC = N // 128        # 16 j-chunks
KC = F // 128        # 2 contraction chunks for h
HALF = N // 2
_CACHE = {}


def _build():
    if "nc" in _CACHE:
        return _CACHE["nc"]

    from contextlib import ExitStack
    import concourse.bacc as bacc
    import concourse.tile as tile
    import concourse.mybir as mybir

    dt = mybir.dt
    AF = mybir.ActivationFunctionType
    ALU = mybir.AluOpType

    nc = bacc.Bacc("TRN2", target_bir_lowering=False, debug=False, num_devices=B)

    xT = nc.dram_tensor("xT", [F, N], dt.float32r, kind="ExternalInput").ap()
    Wsd = nc.dram_tensor("Wsd", [F, F + 2], dt.float32r, kind="ExternalInput").ap()
    Wrep = nc.dram_tensor("Wrep", [F, 128], dt.float32r, kind="ExternalInput").ap()
    maskT = nc.dram_tensor("maskT", [N, N], dt.bfloat16, kind="ExternalInput").ap()
    y = nc.dram_tensor("y", [N, F], dt.float32, kind="ExternalOutput").ap()


    with tile.TileContext(nc) as tc, ExitStack() as ctx:
        sb = ctx.enter_context(tc.tile_pool(name="sb", bufs=1))
        tpool = ctx.enter_context(tc.tile_pool(name="tp", bufs=6))
        p0pool = ctx.enter_context(tc.tile_pool(name="p0", bufs=4))
        phpool = ctx.enter_context(tc.tile_pool(name="ph", bufs=8))
        ypool = ctx.enter_context(tc.tile_pool(name="ysb", bufs=2))
        spool = ctx.enter_context(tc.tile_pool(name="small", bufs=4))
        ps = ctx.enter_context(tc.tile_pool(name="ps", bufs=8, space="PSUM"))

        # ---- persistent SBUF tensors ----
        xT_q = [
            sb.tile([128, KC, 512], dt.float32r, tag=f"xT{i}", name=f"xT{i}")
            for i in range(4)
        ]
        Wsd_t = sb.tile([128, KC, F + 2], dt.float32r, tag="Wsd")
        Wrep_t = sb.tile([128, KC, 128], dt.float32r, tag="Wrep")
        maskS = [
            sb.tile([128, HALF], dt.bfloat16, tag=f"maskS{j}", name=f"maskS{j}")
            for j in range(2 * PC)
        ]  # index w*PC + k -> strip k, i-half w
        hh = sb.tile([128, PC, F + 2], dt.float16, tag="hh")    # [h | 1] per chunk
        Sneg = [
            sb.tile([128, HALF], dt.float16, tag=f"Sneg{i}", name=f"Sneg{i}")
            for i in range(2)
        ]  # -0.8*s_src replicated, per i-half
        c2 = [
            sb.tile([128, 8], dt.float32, tag=f"c2_{i}", name=f"c2_{i}")
            for i in range(2)
        ]  # 0.8*s_dst, chunks 0-7 / 8-15
        bias2 = [
            sb.tile([128, 8], dt.float32, tag=f"bias2_{i}", name=f"bias2_{i}")
            for i in range(2)
        ]  # 0.2*s_dst - 54

        nc.vector.memset(hh[:, :, F : F + 1], 1.0)

        # ---- x loads pipelined with h-matmuls; s_src replication per segment ----
        def emit_seg_mm(seg):
            # replicated s_src segment: [128, 512]
            rp = ps.tile([128, 512], dt.float32, tag="bank", name=f"rep{seg}")
            for c in range(KC):
                nc.tensor.matmul(
                    rp[:],
                    Wrep_t[:, c, :],
                    xT_q[seg][:, c, :],
                    start=(c == 0),
                    stop=(c == KC - 1),
                )
            return rp

        def emit_seg_drain(seg, rp, on_act=False):
            half, off = divmod(seg * 512, HALF)
            dst = Sneg[half][:, off : off + 512]
            if on_act:
                nc.scalar.mul(dst, rp[:], -0.8)
            else:
                nc.vector.tensor_scalar_mul(dst, rp[:], -0.8)

        def emit_seg(seg):
            emit_seg_drain(seg, emit_seg_mm(seg))

        xTr = xT.rearrange("(c p) n -> p c n", p=128)

        def load_mask(w, j):
            nc.sync.dma_start(
                maskS[w * PC + j][:],
                maskT[j * 128 : (j + 1) * 128, w * HALF : (w + 1) * HALF],
            )

        # DMA order tuned so each consumer's data lands just ahead of its use:
        # x quarters feed the preamble matmuls, wave-1 masks interleave behind
        nc.sync.dma_start(xT_q[0][:], xTr[:, :, 0:512])
        nc.sync.dma_start(Wsd_t[:], Wsd.rearrange("(c p) m -> p c m", p=128))
        nc.sync.dma_start(Wrep_t[:], Wrep.rearrange("(c p) m -> p c m", p=128))
        nc.sync.dma_start(xT_q[1][:], xTr[:, :, 512:1024])
        load_mask(0, 0)
        load_mask(0, 1)
        nc.sync.dma_start(xT_q[2][:], xTr[:, :, 1024:1536])
        load_mask(0, 2)
        load_mask(0, 3)
        nc.sync.dma_start(xT_q[3][:], xTr[:, :, 1536:2048])
        for j in range(4, PC):
            load_mask(0, j)
        for j in range(PC):
            load_mask(1, j)
        def emit_h_mm(n_):
            q, off = divmod(n_ * 128, 512)
            hb = ps.tile([128, F + 2], dt.float32, tag="bank", name=f"hb{n_}")
            for c in range(KC):
                nc.tensor.matmul(
                    hb[:],
                    xT_q[q][:, c, off : off + 128],
                    Wsd_t[:, c, :],
                    start=(c == 0),
                    stop=(c == KC - 1),
                )
            return hb

        def emit_h_drain(n_, hb, on_act):
            if on_act:
                nc.scalar.copy(hh[:, n_, 0:F], hb[:, 0:F])
            else:
                nc.vector.tensor_copy(hh[:, n_, 0:F], hb[:, 0:F])
            g, col = divmod(n_, 8)
            nc.vector.tensor_scalar_mul(c2[g][:, col : col + 1], hb[:, F : F + 1], 0.8)
            nc.vector.tensor_scalar(
                bias2[g][:, col : col + 1], hb[:, F : F + 1], 0.2, -54.0,
                op0=ALU.mult, op1=ALU.add,
            )

        hb_late = {}
        for n_ in range(PC):
            hb = emit_h_mm(n_)
            if n_ < 8:
                emit_h_drain(n_, hb, on_act=True)
            else:
                hb_late[n_] = hb
            if n_ == 3:
                emit_seg(0)
            elif n_ == 7:
                emit_seg(1)
        rp_late = {seg: emit_seg_mm(seg) for seg in (2, 3)}

        # ---- normalize + store (staged; one output DMA per wave) ----
        def emit_norm(ysb, sl, bank, on_act):
            rec = spool.tile([128, 1], dt.float32, tag="rec")
            nc.vector.reciprocal(rec[:], bank[:, F : F + 1])
            if on_act:
                nc.scalar.activation(ysb[:, sl, :], bank[:, 0:F], AF.Copy, bias=0.0, scale=rec[:, 0:1])
            else:
                nc.vector.tensor_scalar_mul(ysb[:, sl, :], bank[:, 0:F], rec[:, 0:1])

        # ---- two waves over i-halves; strips over j-chunks.
        # Chunk 8-15 preamble drains are woven into wave-1's early strips so
        # the exp stream starts as soon as the first x quarter lands. Wave-2's
        # first score strips are emitted before wave-1's norms so ACT/DVE keep
        # streaming through the wave boundary while the norms wait on the
        # final wave-1 matmuls.
        def make_ts(w, k):
            g, col = divmod(k, 8)
            t = tpool.tile([128, HALF], dt.float16, tag="t", name=f"t{w}_{k}")
            nc.vector.tensor_scalar(
                t[:], Sneg[w][:], c2[g][:, col : col + 1], bias2[g][:, col : col + 1],
                op0=ALU.max, op1=ALU.add,
            )
            return t

        def make_scores(w, k, t):
            p0 = p0pool.tile([128, HALF], dt.bfloat16, tag="p0", name=f"p0_{w}_{k}")
            nc.scalar.activation(p0[:], t[:], AF.Exp, bias=0.0, scale=1.0)
            ph = phpool.tile([128, HALF], dt.bfloat16, tag="ph", name=f"ph{w}_{k}")
            nc.vector.tensor_mul(ph[:], p0[:], maskS[w * PC + k][:])
            return ph

        def emit_mms(banks, ph, k):
            for ic in range(8):
                nc.tensor.matmul(
                    banks[ic][:, 0 : F + 1],
                    ph[:, ic * 128 : (ic + 1) * 128],
                    hh[:, k, 0 : F + 1],
                    start=(k == 0),
                    stop=(k == PC - 1),
                )

        def emit_norms(w, ybanks, i0):
            for hlf in range(2):
                ysb = ypool.tile([128, 4, F], dt.float32, tag="ysb", name=f"ysb{w}_{hlf}")
                for ic in range(4):
                    g = hlf * 4 + ic
                    emit_norm(ysb, ic, ybanks[g], on_act=(g % 2 == 0))
                lo = i0 + hlf * 512
                nc.sync.dma_start(
                    y[lo : lo + 512, :].rearrange("(c p) f -> p c f", p=128), ysb[:]
                )

        # wave 1
        ybanks1 = [
            ps.tile([128, F + 2], dt.float32, tag="bank", name=f"yb0_{i}")
            for i in range(8)
        ]
        t_next = make_ts(0, 0)
        for k in range(PC):
            t = t_next
            if k + 1 < PC:
                pass
            p0 = p0pool.tile([128, HALF], dt.bfloat16, tag="p0", name=f"p0_0_{k}")
            nc.scalar.activation(p0[:], t[:], AF.Exp, bias=0.0, scale=1.0)
            if k + 1 < PC:
                t_next = make_ts(0, k + 1)
            ph = phpool.tile([128, HALF], dt.bfloat16, tag="ph", name=f"ph0_{k}")
            nc.vector.tensor_mul(ph[:], p0[:], maskS[k][:])
            if k in (0, 1):
                emit_seg_drain(k + 2, rp_late.pop(k + 2), on_act=True)
            if (k + 6) in hb_late:
                emit_h_drain(k + 6, hb_late.pop(k + 6), on_act=False)
            emit_mms(ybanks1, ph, k)

        # wave-2 head scores (pre-emitted across the boundary)
        ybanks2 = [
            ps.tile([128, F + 2], dt.float32, tag="bank", name=f"yb1_{i}")
            for i in range(8)
        ]
        HEAD = 4
        t2 = make_ts(1, 0)
        ph_head = []
        for k in range(HEAD):
            t = t2
            p0 = p0pool.tile([128, HALF], dt.bfloat16, tag="p0", name=f"p0_1_{k}")
            nc.scalar.activation(p0[:], t[:], AF.Exp, bias=0.0, scale=1.0)
            t2 = make_ts(1, k + 1)
            ph = phpool.tile([128, HALF], dt.bfloat16, tag="ph", name=f"ph1_{k}")
            nc.vector.tensor_mul(ph[:], p0[:], maskS[PC + k][:])
            ph_head.append(ph)

        emit_norms(0, ybanks1, 0)

        # wave 2 body
        for k in range(PC):
            if k < HEAD:
                ph = ph_head[k]
            else:
                t = t2
                p0 = p0pool.tile([128, HALF], dt.bfloat16, tag="p0", name=f"p0_1_{k}")
                nc.scalar.activation(p0[:], t[:], AF.Exp, bias=0.0, scale=1.0)
                if k + 1 < PC:
                    t2 = make_ts(1, k + 1)
                ph = phpool.tile([128, HALF], dt.bfloat16, tag="ph", name=f"ph1_{k}")
                nc.vector.tensor_mul(ph[:], p0[:], maskS[PC + k][:])
            emit_mms(ybanks2, ph, k)
        emit_norms(1, ybanks2, HALF)

    nc.compile()
    _CACHE["nc"] = nc
    return nc


def _prep_inputs(x, A, W, a):
    """Host-side layout transforms (per batch element)."""
    import ml_dtypes

    W32 = np.asarray(W, dtype=np.float32)
    a32 = np.asarray(a, dtype=np.float32)
    w_src = W32 @ a32[:F]
    w_dst = W32 @ a32[F:]
    Wsd = np.ascontiguousarray(
        np.concatenate([W32, w_dst[:, None], np.zeros((F, 1), np.float32)], axis=1),
        dtype=np.float32,
    )
    Wrep = np.ascontiguousarray(np.tile(w_src[:, None], (1, 128)), dtype=np.float32)
    in_maps = []
    for b in range(B):
        xTb = np.ascontiguousarray(np.asarray(x[b], dtype=np.float32).T)
        maskTb = np.ascontiguousarray((np.asarray(A[b]).T > 0).astype(ml_dtypes.bfloat16))
        in_maps.append({"xT": xTb, "Wsd": Wsd, "Wrep": Wrep, "maskT": maskTb})
    return in_maps


def kernel(x, A, W, a):
    from concourse.bass_utils import run_bass_kernel_spmd

    nc = _build()
    in_maps = _prep_inputs(x, A, W, a)
    res = run_bass_kernel_spmd(nc, in_maps, list(range(B)))
    out = np.stack([res.results[b]["y"] for b in range(B)]).astype(np.float32)
    return out



# revision 2
# speedup vs baseline: 1.0050x; 1.0050x over previous
"""GATv2-style masked attention kernel for Trainium2, 8-core data-parallel over batch.

Per core (one batch element, N=2048 nodes, F=256 features):
  h = x @ W;  s_src = h @ a[:F], s_dst = h @ a[F:]  (PE, fp32r, fused)
  e[i,j] = leaky_relu(s_src[i] + s_dst[j], 0.2), masked by A
  alpha = softmax_j(e); y = alpha @ h

Softmax without row maxima (per-i factors cancel in y = num/Z):
  P[j,i] = exp(leaky(u) - s_src_i - 54) = max(Ed_j, Es_i) * r_j
with Ed = exp(0.8*s_dst), Es = exp(-0.8*s_src), r = exp(0.2*s_dst - 54) —
exp only touches VECTOR quantities.  Per N x N strip: one tensor_scalar
(max Ed_j, mult r_j) alternating DVE (4x rate) / Pool, then the mask
multiply on DVE (2x rate).

The i range runs in FOUR waves of 4 PSUM banks ([128, 258] fp32 each burns a
full 2-KB bank; 4 live + 4 draining fit the 8 banks so wave w+1 never stalls
on wave w's normalization).  Scores are transposed ([j, i]) so P @ h
contracts j on partitions.  The h/s_src preamble matmuls are woven between
wave-0's P@h matmuls so PE streams continuously from the first strip.  Norms
split 2/2 across ACT and DVE; one fp16 output DMA per wave, interleaved with
the later waves' mask loads on the DMA queue.  Host supplies x transposed,
mask transposed bf16, [W | W@a_dst | 0], and W@a_src replicated across 128
columns (pure layout/weight transforms).
"""

import numpy as np

B, N, F = 8, 2048, 256
PC = N // 128        # 16 j-chunks
KC = F // 128        # 2 contraction chunks for h
WAVE = 512           # i-columns per wave
NW = N // WAVE       # 4 waves
_CACHE = {}


def _build():
    if "nc" in _CACHE:
        return _CACHE["nc"]

    from contextlib import ExitStack
    import concourse.bacc as bacc
    import concourse.tile as tile
    import concourse.mybir as mybir

    dt = mybir.dt
    AF = mybir.ActivationFunctionType
    ALU = mybir.AluOpType

    nc = bacc.Bacc("TRN2", target_bir_lowering=False, debug=False, num_devices=B)

    xT = nc.dram_tensor("xT", [F, N], dt.float16, kind="ExternalInput").ap()
    Wsd = nc.dram_tensor("Wsd", [F, F + 2], dt.float16, kind="ExternalInput").ap()
    Wrep = nc.dram_tensor("Wrep", [F, 128], dt.float16, kind="ExternalInput").ap()
    maskT = nc.dram_tensor("maskT", [N, N], dt.bfloat16, kind="ExternalInput").ap()
    y = nc.dram_tensor("y", [N, F], dt.float16, kind="ExternalOutput").ap()

    with tile.TileContext(nc) as tc, ExitStack() as ctx:
        sb = ctx.enter_context(tc.tile_pool(name="sb", bufs=1))
        p0pool = ctx.enter_context(tc.tile_pool(name="p0", bufs=8))
        phpool = ctx.enter_context(tc.tile_pool(name="ph", bufs=8))
        ypool = ctx.enter_context(tc.tile_pool(name="ysb", bufs=2))
        spool = ctx.enter_context(tc.tile_pool(name="small", bufs=8))
        ps = ctx.enter_context(tc.tile_pool(name="ps", bufs=8, space="PSUM"))

        # ---- persistent SBUF tensors ----
        xT_q = [
            sb.tile([128, KC, 512], dt.float16, tag=f"xT{i}", name=f"xT{i}")
            for i in range(4)
        ]
        Wsd_t = sb.tile([128, KC, F + 2], dt.float16, tag="Wsd")
        Wrep_t = sb.tile([128, KC, 128], dt.float16, tag="Wrep")
        maskQ = [
            [
                sb.tile([128, 4, WAVE], dt.bfloat16, tag=f"mQ{w}_{g}", name=f"mQ{w}_{g}")
                for g in range(4)
            ]
            for w in range(NW)
        ]

        def mask_sl(w, k):
            return maskQ[w][k // 4][:, k % 4, :]

        hh = sb.tile([128, PC, F + 2], dt.float16, tag="hh")    # [h | 1] per chunk
        Es = [
            sb.tile([128, WAVE], dt.bfloat16, tag=f"Es{i}", name=f"Es{i}")
            for i in range(NW)
        ]  # exp(-0.8*s_src) replicated, per wave
        Edc = [
            sb.tile([128, 1], dt.float32, tag=f"Edc{k}", name=f"Edc{k}")
            for k in range(PC)
        ]  # exp(0.8*s_dst) per j-chunk
        rc = [
            sb.tile([128, 1], dt.float32, tag=f"rc{k}", name=f"rc{k}")
            for k in range(PC)
        ]  # exp(0.2*s_dst - 54) per j-chunk

        biasC = sb.tile([128, 1], dt.float32, tag="biasC")
        nc.vector.memset(biasC[:], -54.0)
        nc.vector.memset(hh[:, :, F : F + 1], 1.0)

        # ---- upfront DMA queue: weights, x quarters, wave-0/1 masks ----
        xTr = xT.rearrange("(c p) n -> p c n", p=128)
        maskR = maskT.rearrange("(g c p) n -> g p c n", p=128, c=4)

        def load_mask(w, g):
            nc.sync.dma_start(
                maskQ[w][g][:],
                maskR[g, :, :, w * WAVE : (w + 1) * WAVE],
            )

        # Wrep + x0 land first: they gate seg0 -> Es[0] -> the whole score
        # stream.  Wsd (h-matmuls) next, then wave-0 masks interleaved with
        # the remaining x quarters, then the later waves' masks.
        nc.sync.dma_start(Wrep_t[:], Wrep.rearrange("(c p) m -> p c m", p=128))
        nc.sync.dma_start(xT_q[0][:], xTr[:, :, 0:512])
        nc.sync.dma_start(Wsd_t[:], Wsd.rearrange("(c p) m -> p c m", p=128))
        load_mask(0, 0)
        nc.sync.dma_start(xT_q[1][:], xTr[:, :, 512:1024])
        load_mask(0, 1)
        nc.sync.dma_start(xT_q[2][:], xTr[:, :, 1024:1536])
        load_mask(0, 2)
        nc.sync.dma_start(xT_q[3][:], xTr[:, :, 1536:2048])
        load_mask(0, 3)
        for w in range(1, NW):
            for g in range(4):
                load_mask(w, g)

        # ---- preamble pieces (emitted inline / woven into wave 0) ----
        def emit_seg_mm(seg):
            rp = ps.tile([128, 512], dt.float32, tag="bank", name=f"rep{seg}")
            for c in range(KC):
                nc.tensor.matmul(
                    rp[:],
                    Wrep_t[:, c, :],
                    xT_q[seg][:, c, :],
                    start=(c == 0),
                    stop=(c == KC - 1),
                )
            nc.scalar.activation(Es[seg][:], rp[:], AF.Exp, bias=0.0, scale=-0.8)

        def emit_h(n_):
            q, off = divmod(n_ * 128, 512)
            hb = ps.tile([128, F + 2], dt.float32, tag="bank", name=f"hb{n_}")
            for c in range(KC):
                nc.tensor.matmul(
                    hb[:],
                    xT_q[q][:, c, off : off + 128],
                    Wsd_t[:, c, :],
                    start=(c == 0),
                    stop=(c == KC - 1),
                )
            # per-chunk scalar exps first (gate the score stream), then h copy
            nc.scalar.activation(Edc[n_][:], hb[:, F : F + 1], AF.Exp, bias=0.0, scale=0.8)
            nc.scalar.activation(rc[n_][:], hb[:, F : F + 1], AF.Exp, bias=biasC[:, 0:1], scale=0.2)
            nc.scalar.copy(hh[:, n_, 0:F], hb[:, 0:F])

        # ---- score strip: p0 = max(Es, Ed_j) * r_j (DVE 4x / Pool alt),
        # ph = p0 * mask (DVE 2x) ----
        def make_scores(w, k):
            p0 = p0pool.tile([128, WAVE], dt.bfloat16, tag="p0", name=f"p0_{w}_{k}")
            eng = nc.vector if (w * PC + k) % 2 == 0 else nc.gpsimd
            eng.tensor_scalar(
                p0[:], Es[w][:], Edc[k][:, 0:1], rc[k][:, 0:1],
                op0=ALU.max, op1=ALU.mult,
            )
            ph = phpool.tile([128, WAVE], dt.bfloat16, tag="ph", name=f"ph{w}_{k}")
            nc.vector.tensor_mul(ph[:], p0[:], mask_sl(w, k))
            return ph

        def emit_mms(banks, ph, k):
            for ic in range(4):
                nc.tensor.matmul(
                    banks[ic][:, 0 : F + 1],
                    ph[:, ic * 128 : (ic + 1) * 128],
                    hh[:, k, 0 : F + 1],
                    start=(k == 0),
                    stop=(k == PC - 1),
                )

        # tiny keep-alive matmul: holds the PE busy-streak (p-state ramp)
        # through known strip-arrival gaps in wave 0
        def keep_alive(i):
            kb = ps.tile([128, 1], dt.float32, tag="bank", name=f"ka{i}")
            nc.tensor.matmul(
                kb[:],
                Wrep_t[:, 0, 0:128],
                Wsd_t[:, 0, 0:1],
                start=True,
                stop=True,
            )

        # ---- normalize + store ----
        def emit_norm(ysb, sl, bank, on_act):
            rec = spool.tile([128, 1], dt.float32, tag="rec")
            nc.vector.reciprocal(rec[:], bank[:, F : F + 1])
            if on_act:
                nc.scalar.activation(
                    ysb[:, sl, :], bank[:, 0:F], AF.Copy, bias=0.0, scale=rec[:, 0:1]
                )
            else:
                nc.vector.tensor_scalar_mul(ysb[:, sl, :], bank[:, 0:F], rec[:, 0:1])

        def emit_norms(w, banks):
            # norm copies on ACT (DVE is the score-stream pacer, ACT idles),
            # except the final wave where DVE is free and 2/2 halves the
            # serial norm chain on the tail
            ysb = ypool.tile([128, 4, F], dt.float16, tag="ysb", name=f"ysb{w}")
            for ic in range(4):
                emit_norm(ysb, ic, banks[ic], on_act=(w < NW - 1 or ic % 2 == 0))
            lo = w * WAVE
            # y stores issue from the ACT sequencer so the SP queue keeps
            # streaming mask loads without blocking on norm completion; the
            # final wave stores per-bank from the (by-then idle) SP queue so
            # the last transfer is small and leaves as early as possible
            if w == NW - 1:
                for hf in range(2):
                    nc.sync.dma_start(
                        y[lo + hf * 256 : lo + (hf + 1) * 256, :].rearrange(
                            "(c p) f -> p c f", p=128
                        ),
                        ysb[:, 2 * hf : 2 * hf + 2, :],
                    )
            else:
                nc.scalar.dma_start(
                    y[lo : lo + WAVE, :].rearrange("(c p) f -> p c f", p=128), ysb[:]
                )

        # ---- wave 0 with preamble woven between its P@h matmuls.
        # weave[k]: preamble unit emitted after wave-0's k-th strip matmuls.
        # h8-11 need x quarter 2, h12-15 + seg2/3 need quarter 3 — placed so
        # PE reaches them after their DMA lands.
        emit_seg_mm(0)
        for n_ in range(4):
            emit_h(n_)

        weave = {0: lambda: emit_h(4), 1: lambda: emit_h(5),
                 2: lambda: emit_seg_mm(1),
                 3: lambda: emit_h(6), 4: lambda: emit_h(7),
                 5: lambda: emit_h(8), 6: lambda: emit_h(9),
                 7: lambda: emit_h(10), 8: lambda: emit_h(11),
                 9: lambda: emit_seg_mm(2),
                 10: lambda: emit_h(12), 11: lambda: emit_h(13),
                 12: lambda: emit_h(14), 13: lambda: emit_h(15),
                 14: lambda: emit_seg_mm(3)}

        HEAD = 4
        pending = [make_scores(0, k) for k in range(HEAD)]
        nka = 0
        for w in range(NW):
            banks = [
                ps.tile([128, F + 2], dt.float32, tag="bank", name=f"yb{w}_{i}")
                for i in range(4)
            ]
            for k in range(PC):
                ph = pending.pop(0) if k < HEAD else make_scores(w, k)
                if k >= PC - HEAD and w + 1 < NW:
                    pending.append(make_scores(w + 1, k - (PC - HEAD)))
                emit_mms(banks, ph, k)
                if w == 0 and k in weave:
                    weave[k]()
                if w == 0 and k in (2, 3, 4, 5, 6):
                    keep_alive(nka)
                    nka += 1
            emit_norms(w, banks)

    nc.compile()
    _CACHE["nc"] = nc
    return nc


def _prep_inputs(x, A, W, a):
    """Host-side layout transforms (per batch element)."""
    import ml_dtypes

    W32 = np.asarray(W, dtype=np.float32)
    a32 = np.asarray(a, dtype=np.float32)
    w_src = W32 @ a32[:F]
    w_dst = W32 @ a32[F:]
    Wsd = np.ascontiguousarray(
        np.concatenate([W32, w_dst[:, None], np.zeros((F, 1), np.float32)], axis=1),
        dtype=np.float32,
    ).astype(np.float16)
    Wrep = np.ascontiguousarray(np.tile(w_src[:, None], (1, 128)), dtype=np.float16)
    in_maps = []
    for b in range(B):
        xTb = np.ascontiguousarray(np.asarray(x[b], dtype=np.float32).T.astype(np.float16))
        maskTb = np.ascontiguousarray((np.asarray(A[b]).T > 0).astype(ml_dtypes.bfloat16))
        in_maps.append({"xT": xTb, "Wsd": Wsd, "Wrep": Wrep, "maskT": maskTb})
    return in_maps


def kernel(x, A, W, a):
    from concourse.bass_utils import run_bass_kernel_spmd

    nc = _build()
    in_maps = _prep_inputs(x, A, W, a)
    res = run_bass_kernel_spmd(nc, in_maps, list(range(B)))
    out = np.stack([res.results[b]["y"] for b in range(B)]).astype(np.float32)
    return out


# revision 3
# speedup vs baseline: 1.0144x; 1.0093x over previous
"""GATv2-style masked attention kernel for Trainium2, 8-core data-parallel over batch.

Per core (one batch element, N=2048 nodes, F=256 features):
  h = x @ W;  s_src = h @ a[:F], s_dst = h @ a[F:]  (PE, fp32r, fused)
  e[i,j] = leaky_relu(s_src[i] + s_dst[j], 0.2), masked by A
  alpha = softmax_j(e); y = alpha @ h

Softmax without row maxima (per-i factors cancel in y = num/Z):
  P[j,i] = exp(leaky(u) - s_src_i - 54) = max(Ed_j, Es_i) * r_j
with Ed = exp(0.8*s_dst), Es = exp(-0.8*s_src), r = exp(0.2*s_dst - 54) —
exp only touches VECTOR quantities.  Per N x N strip: one tensor_scalar
(max Ed_j, mult r_j) alternating DVE (4x rate) / Pool, then the mask
multiply on DVE (2x rate).

The i range runs in FOUR waves of 4 PSUM banks ([128, 258] fp32 each burns a
full 2-KB bank; 4 live + 4 draining fit the 8 banks so wave w+1 never stalls
on wave w's normalization).  Scores are transposed ([j, i]) so P @ h
contracts j on partitions.  The h/s_src preamble matmuls are woven between
wave-0's P@h matmuls so PE streams continuously from the first strip.  Norms
split 2/2 across ACT and DVE; one fp16 output DMA per wave, interleaved with
the later waves' mask loads on the DMA queue.  Host supplies x transposed,
mask transposed bf16, [W | W@a_dst | 0], and W@a_src replicated across 128
columns (pure layout/weight transforms).
"""

import numpy as np

B, N, F = 8, 2048, 256
PC = N // 128        # 16 j-chunks
KC = F // 128        # 2 contraction chunks for h
WAVE = 512           # i-columns per wave
NW = N // WAVE       # 4 waves
_CACHE = {}


def _build():
    if "nc" in _CACHE:
        return _CACHE["nc"]

    from contextlib import ExitStack
    import concourse.bacc as bacc
    import concourse.tile as tile
    import concourse.mybir as mybir

    dt = mybir.dt
    AF = mybir.ActivationFunctionType
    ALU = mybir.AluOpType

    nc = bacc.Bacc("TRN2", target_bir_lowering=False, debug=False, num_devices=B)

    xT = nc.dram_tensor("xT", [F, N], dt.float16, kind="ExternalInput").ap()
    Wsd = nc.dram_tensor("Wsd", [F, F + 2], dt.float16, kind="ExternalInput").ap()
    Wrep = nc.dram_tensor("Wrep", [F, 128], dt.float16, kind="ExternalInput").ap()
    maskT = nc.dram_tensor("maskT", [N, N], dt.bfloat16, kind="ExternalInput").ap()
    y = nc.dram_tensor("y", [N, F], dt.float16, kind="ExternalOutput").ap()

    with tile.TileContext(nc) as tc, ExitStack() as ctx:
        sb = ctx.enter_context(tc.tile_pool(name="sb", bufs=1))
        p0pool = ctx.enter_context(tc.tile_pool(name="p0", bufs=8))
        phpool = ctx.enter_context(tc.tile_pool(name="ph", bufs=8))
        ypool = ctx.enter_context(tc.tile_pool(name="ysb", bufs=2))
        spool = ctx.enter_context(tc.tile_pool(name="small", bufs=8))
        ps = ctx.enter_context(tc.tile_pool(name="ps", bufs=8, space="PSUM"))

        # ---- persistent SBUF tensors ----
        xT_q = [
            sb.tile([128, KC, 512], dt.float16, tag=f"xT{i}", name=f"xT{i}")
            for i in range(4)
        ]
        Wsd_t = sb.tile([128, KC, F + 2], dt.float16, tag="Wsd")
        Wrep_t = sb.tile([128, KC, 128], dt.float16, tag="Wrep")
        maskQ = [
            [
                sb.tile([128, 4, WAVE], dt.bfloat16, tag=f"mQ{w}_{g}", name=f"mQ{w}_{g}")
                for g in range(4)
            ]
            for w in range(NW)
        ]

        def mask_sl(w, k):
            return maskQ[w][k // 4][:, k % 4, :]

        hh = sb.tile([128, PC, F + 2], dt.float16, tag="hh")    # [h | 1] per chunk
        Es = [
            sb.tile([128, WAVE], dt.bfloat16, tag=f"Es{i}", name=f"Es{i}")
            for i in range(NW)
        ]  # exp(-0.8*s_src) replicated, per wave
        Edc = [
            sb.tile([128, 1], dt.float32, tag=f"Edc{k}", name=f"Edc{k}")
            for k in range(PC)
        ]  # exp(0.8*s_dst) per j-chunk
        rc = [
            sb.tile([128, 1], dt.float32, tag=f"rc{k}", name=f"rc{k}")
            for k in range(PC)
        ]  # exp(0.2*s_dst - 54) per j-chunk

        biasC = sb.tile([128, 1], dt.float32, tag="biasC")
        nc.vector.memset(biasC[:], -54.0)
        nc.vector.memset(hh[:, :, F : F + 1], 1.0)

        # ---- upfront DMA queue: weights, x quarters, wave-0/1 masks ----
        xTr = xT.rearrange("(c p) n -> p c n", p=128)
        maskR = maskT.rearrange("(g c p) n -> g p c n", p=128, c=4)

        def load_mask(w, g):
            nc.sync.dma_start(
                maskQ[w][g][:],
                maskR[g, :, :, w * WAVE : (w + 1) * WAVE],
            )

        # Wrep + x0 land first: they gate seg0 -> Es[0] -> the whole score
        # stream.  Wsd (h-matmuls) next, then wave-0 masks interleaved with
        # the remaining x quarters, then the later waves' masks.
        nc.sync.dma_start(Wrep_t[:], Wrep.rearrange("(c p) m -> p c m", p=128))
        nc.sync.dma_start(xT_q[0][:], xTr[:, :, 0:512])
        nc.sync.dma_start(Wsd_t[:], Wsd.rearrange("(c p) m -> p c m", p=128))
        # first mask group split 2+2 so strip 0's multiply unblocks ~0.4us
        # sooner (its transfer is the longest link in the head chain)
        nc.sync.dma_start(maskQ[0][0][:, 0:2, :], maskR[0, :, 0:2, 0:WAVE])
        nc.sync.dma_start(maskQ[0][0][:, 2:4, :], maskR[0, :, 2:4, 0:WAVE])
        nc.sync.dma_start(xT_q[1][:], xTr[:, :, 512:1024])
        load_mask(0, 1)
        nc.sync.dma_start(xT_q[2][:], xTr[:, :, 1024:1536])
        load_mask(0, 2)
        nc.sync.dma_start(xT_q[3][:], xTr[:, :, 1536:2048])
        load_mask(0, 3)
        for w in range(1, NW):
            for g in range(4):
                load_mask(w, g)

        # ---- preamble pieces (emitted inline / woven into wave 0) ----
        def emit_seg_mm(seg):
            rp = ps.tile([128, 512], dt.float32, tag="bank", name=f"rep{seg}")
            for c in range(KC):
                nc.tensor.matmul(
                    rp[:],
                    Wrep_t[:, c, :],
                    xT_q[seg][:, c, :],
                    start=(c == 0),
                    stop=(c == KC - 1),
                )
            nc.scalar.activation(Es[seg][:], rp[:], AF.Exp, bias=0.0, scale=-0.8)

        def emit_h(n_):
            q, off = divmod(n_ * 128, 512)
            hb = ps.tile([128, F + 2], dt.float32, tag="bank", name=f"hb{n_}")
            for c in range(KC):
                nc.tensor.matmul(
                    hb[:],
                    xT_q[q][:, c, off : off + 128],
                    Wsd_t[:, c, :],
                    start=(c == 0),
                    stop=(c == KC - 1),
                )
            # per-chunk scalar exps first (gate the score stream), then h copy
            nc.scalar.activation(Edc[n_][:], hb[:, F : F + 1], AF.Exp, bias=0.0, scale=0.8)
            nc.scalar.activation(rc[n_][:], hb[:, F : F + 1], AF.Exp, bias=biasC[:, 0:1], scale=0.2)
            nc.scalar.copy(hh[:, n_, 0:F], hb[:, 0:F])

        # ---- score strip: p0 = max(Es, Ed_j) * r_j (DVE 4x / Pool alt),
        # ph = p0 * mask (DVE 2x) ----
        def make_scores(w, k):
            p0 = p0pool.tile([128, WAVE], dt.bfloat16, tag="p0", name=f"p0_{w}_{k}")
            eng = nc.vector if (w * PC + k) % 2 == 0 else nc.gpsimd
            eng.tensor_scalar(
                p0[:], Es[w][:], Edc[k][:, 0:1], rc[k][:, 0:1],
                op0=ALU.max, op1=ALU.mult,
            )
            ph = phpool.tile([128, WAVE], dt.bfloat16, tag="ph", name=f"ph{w}_{k}")
            nc.vector.tensor_mul(ph[:], p0[:], mask_sl(w, k))
            return ph

        def emit_mms(banks, ph, k):
            for ic in range(4):
                nc.tensor.matmul(
                    banks[ic][:, 0 : F + 1],
                    ph[:, ic * 128 : (ic + 1) * 128],
                    hh[:, k, 0 : F + 1],
                    start=(k == 0),
                    stop=(k == PC - 1),
                )

        # tiny keep-alive matmul: holds the PE busy-streak (p-state ramp)
        # through known strip-arrival gaps in wave 0
        def keep_alive(i):
            kb = ps.tile([128, 1], dt.float32, tag="bank", name=f"ka{i}")
            nc.tensor.matmul(
                kb[:],
                Wrep_t[:, 0, 0:128],
                Wsd_t[:, 0, 0:1],
                start=True,
                stop=True,
            )

        # ---- normalize + store ----
        def emit_norm(ysb, sl, bank, on_act):
            rec = spool.tile([128, 1], dt.float32, tag="rec")
            nc.vector.reciprocal(rec[:], bank[:, F : F + 1])
            if on_act:
                nc.scalar.activation(
                    ysb[:, sl, :], bank[:, 0:F], AF.Copy, bias=0.0, scale=rec[:, 0:1]
                )
            else:
                nc.vector.tensor_scalar_mul(ysb[:, sl, :], bank[:, 0:F], rec[:, 0:1])

        def emit_norms(w, banks):
            # norm copies on ACT (DVE is the score-stream pacer, ACT idles),
            # except the final wave where DVE is free and 2/2 halves the
            # serial norm chain on the tail
            ysb = ypool.tile([128, 4, F], dt.float16, tag="ysb", name=f"ysb{w}")
            for ic in range(4):
                emit_norm(ysb, ic, banks[ic], on_act=(w < NW - 1 or ic % 2 == 0))
            lo = w * WAVE
            # y stores issue from the ACT sequencer so the SP queue keeps
            # streaming mask loads without blocking on norm completion; the
            # final wave stores per-bank from the (by-then idle) SP queue so
            # the last transfer is small and leaves as early as possible
            if w == NW - 1:
                for hf in range(2):
                    nc.sync.dma_start(
                        y[lo + hf * 256 : lo + (hf + 1) * 256, :].rearrange(
                            "(c p) f -> p c f", p=128
                        ),
                        ysb[:, 2 * hf : 2 * hf + 2, :],
                    )
            else:
                nc.scalar.dma_start(
                    y[lo : lo + WAVE, :].rearrange("(c p) f -> p c f", p=128), ysb[:]
                )

        # ---- wave 0 with preamble woven between its P@h matmuls.
        # weave[k]: preamble unit emitted after wave-0's k-th strip matmuls.
        # h8-11 need x quarter 2, h12-15 + seg2/3 need quarter 3 — placed so
        # PE reaches them after their DMA lands.
        emit_seg_mm(0)
        for n_ in range(4):
            emit_h(n_)

        # weave invariant: emit_h(n) must sit at slot <= n-1 so the chunk's
        # Edc/hh writers are emitted before any strip that reads them; slots
        # are also placed at/after the PE time their x quarter lands, so a
        # late DMA never blocks ready strip matmuls queued behind it.
        def _h(n):
            return lambda: emit_h(n)

        weave = {3: lambda: (emit_h(4), emit_h(5)),
                 4: _h(6), 5: _h(7),
                 6: lambda: emit_seg_mm(1),
                 7: _h(8), 8: _h(9), 9: _h(10), 10: _h(11),
                 11: _h(12), 12: _h(13), 13: _h(14), 14: _h(15),
                 15: lambda: emit_seg_mm(2)}

        HEAD = 4
        pending = [make_scores(0, k) for k in range(HEAD)]
        nka = 0
        for w in range(NW):
            banks = [
                ps.tile([128, F + 2], dt.float32, tag="bank", name=f"yb{w}_{i}")
                for i in range(4)
            ]
            for k in range(PC):
                ph = pending.pop(0) if k < HEAD else make_scores(w, k)
                if k >= PC - HEAD and w + 1 < NW:
                    pending.append(make_scores(w + 1, k - (PC - HEAD)))
                emit_mms(banks, ph, k)
                if w == 0 and k in weave:
                    weave[k]()
                if w == 1 and k == 0:
                    emit_seg_mm(3)
                if w == 0 and k in (0, 1, 2, 5, 6):
                    keep_alive(nka)
                    nka += 1
            emit_norms(w, banks)

    nc.compile()
    _CACHE["nc"] = nc
    return nc


def _prep_inputs(x, A, W, a):
    """Host-side layout transforms (per batch element)."""
    import ml_dtypes

    W32 = np.asarray(W, dtype=np.float32)
    a32 = np.asarray(a, dtype=np.float32)
    w_src = W32 @ a32[:F]
    w_dst = W32 @ a32[F:]
    Wsd = np.ascontiguousarray(
        np.concatenate([W32, w_dst[:, None], np.zeros((F, 1), np.float32)], axis=1),
        dtype=np.float32,
    ).astype(np.float16)
    Wrep = np.ascontiguousarray(np.tile(w_src[:, None], (1, 128)), dtype=np.float16)
    in_maps = []
    for b in range(B):
        xTb = np.ascontiguousarray(np.asarray(x[b], dtype=np.float32).T.astype(np.float16))
        maskTb = np.ascontiguousarray((np.asarray(A[b]).T > 0).astype(ml_dtypes.bfloat16))
        in_maps.append({"xT": xTb, "Wsd": Wsd, "Wrep": Wrep, "maskT": maskTb})
    return in_maps


def kernel(x, A, W, a):
    from concourse.bass_utils import run_bass_kernel_spmd

    nc = _build()
    in_maps = _prep_inputs(x, A, W, a)
    res = run_bass_kernel_spmd(nc, in_maps, list(range(B)))
    out = np.stack([res.results[b]["y"] for b in range(B)]).astype(np.float32)
    return out
